# revision 1
# baseline (speedup 1.0000x reference)
"""AttentionDot kernel v2 for Trainium2 (Bass/Tile), 8-core data-parallel over batch.

Math (per batch b):
    prod[tp,tq,d] = q[tq,d] * p[tp,d]
    scores[tp,tq] = tanh(prod @ W) @ vd
    weights       = softmax(scores over tp)
    out[tp,d]     = sum_tq weights[tp,tq] * q[tq,d]

v2 changes over the baseline:
  - software-pipelined emission: prods+mains for group g+1 are emitted BEFORE
    tanh(g)/deltas(g), so the in-order PE never parks behind tanh-dependent
    delta matmuls while independent mains are ready (was the per-group stall).
  - sc PSUM zero-init matmuls dropped: the first delta per PE column group
    uses start=True instead (flags tracked by emission order per col group).
  - sc_pool bufs=2 so iteration i+1's deltas/mains overlap iteration i's
    softmax/output tail across the For_i back edge.
  - softmax tail: one [128,512] Exp (no accum_out) + one segmented DVE
    tensor_reduce for Z; e_sb/qn in bf16 so the output matmuls run at
    1 cycle/row instead of fp32's 4.
  - single output DMA straight from PSUM (rearranged), no SBUF bounce.
"""

import numpy as np

B, TQ, TP, D = 8, 256, 256, 128
NCORES = 8
NPAIR = TQ // 2  # 128 pairs of tq
GP = 3  # pairs per PSUM staging group (3 banks of 2KB)

_nc_cache = {}


def _build_nc(repeat=1, gp=GP, prod_bufs=6, t_bufs=3, s_bufs=2, exp_mode='accum'):
    from contextlib import ExitStack

    import concourse.bacc as bacc
    import concourse.tile as tile
    from concourse import mybir
    from concourse.masks import make_identity

    f32 = mybir.dt.float32
    bf16 = mybir.dt.bfloat16
    AF = mybir.ActivationFunctionType
    Op = __import__("concourse.alu_op_type", fromlist=["AluOpType"]).AluOpType

    nc = bacc.Bacc("TRN2", target_bir_lowering=False, debug=False)
    q_d = nc.dram_tensor("q", [TQ, D], f32, kind="ExternalInput").ap()
    p_d = nc.dram_tensor("p", [TP, D], f32, kind="ExternalInput").ap()
    w_d = nc.dram_tensor("W", [D, D], f32, kind="ExternalInput").ap()
    vd_d = nc.dram_tensor("vd", [D, 1], f32, kind="ExternalInput").ap()
    out_d = nc.dram_tensor("out", [TP, D], f32, kind="ExternalOutput").ap()

    with tile.TileContext(nc) as tc, ExitStack() as ctx:
        consts = ctx.enter_context(tc.tile_pool(name="consts", bufs=1))
        nat_pool = ctx.enter_context(tc.tile_pool(name="nat", bufs=2))
        prod_pool = ctx.enter_context(tc.tile_pool(name="prod", bufs=prod_bufs))
        t_pool = ctx.enter_context(tc.tile_pool(name="tsb", bufs=t_bufs))
        s_pool = ctx.enter_context(tc.tile_pool(name="sps", bufs=s_bufs, space="PSUM"))
        sc_pool = ctx.enter_context(tc.tile_pool(name="scps", bufs=2, space="PSUM"))

        # ---------------- constants / setup ----------------
        ident = consts.tile([128, 128], f32, name="ident", tag="ident")
        make_identity(nc, ident)

        w_f = consts.tile([D, D], f32, name="w_f", tag="w_f")
        nc.sync.dma_start(w_f, w_d)
        w_bf = consts.tile([D, D], bf16, name="w_bf", tag="w_bf")
        nc.vector.tensor_copy(w_bf, w_f)

        vd_f = consts.tile([D, 1], f32, name="vd_f", tag="vd_f")
        nc.sync.dma_start(vd_f, vd_d)
        # sliding-window delta weights: zeros with vd at column NPAIR-1
        vdw = consts.tile([D, 2 * NPAIR - 1], bf16, name="vdw", tag="vdw")
        nc.vector.memset(vdw, 0.0)
        nc.vector.tensor_copy(vdw[:, NPAIR - 1 : NPAIR], vd_f)

        # q rows interleaved even/odd: partition t holds [q[2t,:] | q[2t+1,:]]
        q_eo = consts.tile([NPAIR, 2 * D], f32, name="q_eo", tag="q_eo")
        nc.sync.dma_start(q_eo, q_d.rearrange("(t two) d -> t (two d)", two=2))

        # transposes: qT [d, tq] (f32, used as per-partition scalars), pT bf16
        qT = consts.tile([D, TQ], f32, name="qT", tag="qT")
        pT = consts.tile([D, TP], bf16, name="pT", tag="pT")
        for src_d, dstT, nm in ((q_d, qT, "q"), (p_d, pT, "p")):
            tr_ps = s_pool.tile([128, gp * 512], f32, name=f"trps_{nm}", tag="s")
            for h in range(2):
                nat = nat_pool.tile([128, D], f32, name=f"nat_{nm}{h}", tag="nat")
                nc.sync.dma_start(nat, src_d[h * 128 : (h + 1) * 128, :])
                nc.tensor.transpose(tr_ps[:, h * 128 : (h + 1) * 128], nat, ident)
            nc.vector.tensor_copy(dstT[:, 0:128], tr_ps[:, 0:128])
            nc.vector.tensor_copy(dstT[:, 128:256], tr_ps[:, 128:256])

        # warm the ACT table set (exp_and_others holds both Tanh and Exp)
        act_warm = consts.tile([128, 1], f32, name="act_warm", tag="warm")
        nc.vector.memset(act_warm, 0.0)
        nc.scalar.activation(act_warm, act_warm, AF.Tanh)
        nc.scalar.activation(act_warm, act_warm, AF.Exp)

        order = [32 * g + c for c in range(32) for g in range(4)]
        groups = [order[k : k + gp] for k in range(0, len(order), gp)]
        NG = len(groups)

        # ---------------- main compute body ----------------
        def body():
            sc_ps = sc_pool.tile([128, 2 * TP], f32, name="sc_ps", tag="sc")
            emitted = [0, 0, 0, 0]

            def emit_pm(gi):
                pairs = groups[gi]
                s_ps = s_pool.tile([128, gp * 512], f32, name=f"s_ps_{gi}", tag="s")
                for k, i in enumerate(pairs):
                    prod = prod_pool.tile(
                        [128, 512], bf16, name=f"prod_{i}", tag="prod"
                    )
                    nc.vector.tensor_scalar_mul(
                        prod[:, 0:TP], pT, qT[:, 2 * i : 2 * i + 1]
                    )
                    nc.vector.tensor_scalar_mul(
                        prod[:, TP : 2 * TP], pT, qT[:, 2 * i + 1 : 2 * i + 2]
                    )
                    nc.tensor.matmul(
                        s_ps[:, k * 512 : (k + 1) * 512],
                        lhsT=w_bf,
                        rhs=prod,
                        start=True,
                        stop=True,
                    )
                return s_ps

            def emit_deltas(gi, t_sb):
                for k, i in enumerate(groups[gi]):
                    g, c = i // 32, i % 32
                    emitted[g] += 1
                    nc.tensor.matmul(
                        sc_ps[32 * g : 32 * (g + 1), :],
                        lhsT=vdw[:, NPAIR - 1 - c : NPAIR - 1 - c + 32],
                        rhs=t_sb[:, k * 512 : (k + 1) * 512],
                        start=(emitted[g] == 1),
                        stop=(emitted[g] == 32),
                        skip_group_check=True,
                        tile_position=(0, 32 * g),
                    )

            s_prev = emit_pm(0)
            for gi in range(NG):
                s_next = emit_pm(gi + 1) if gi + 1 < NG else None
                n = len(groups[gi])
                t_sb = t_pool.tile([128, gp * 512], bf16, name=f"t_{gi}", tag="t")
                nc.scalar.activation(
                    t_sb[:, : n * 512], s_prev[:, : n * 512], AF.Tanh
                )
                emit_deltas(gi, t_sb)
                s_prev = s_next

            # ---------- softmax (over tp, the free axis) + output ----------
            e_sb = consts.tile([128, 2 * TP], bf16, name="e_sb", tag="e_sb")
            z2 = consts.tile([128, 2], f32, name="z2", tag="z2")
            if exp_mode == 'accum':
                # ACT is idle at the tail: inline free-dim accumulation avoids
                # the cross-engine exp -> DVE-reduce handoff
                nc.scalar.activation(
                    e_sb[:, 0:TP], sc_ps[:, 0:TP], AF.Exp, accum_out=z2[:, 0:1]
                )
                nc.scalar.activation(
                    e_sb[:, TP : 2 * TP], sc_ps[:, TP : 2 * TP], AF.Exp,
                    accum_out=z2[:, 1:2],
                )
            else:
                nc.scalar.activation(e_sb, sc_ps, AF.Exp)
                e3 = e_sb[:, :].rearrange("p (h t) -> p h t", h=2)
                nc.vector.tensor_reduce(z2, e3, mybir.AxisListType.X, Op.add)
            rz = consts.tile([128, 2], f32, name="rz", tag="rz")
            nc.vector.reciprocal(rz, z2)
            qn = consts.tile([128, 2 * D], bf16, name="qn", tag="qn")
            nc.vector.tensor_scalar_mul(qn[:, 0:D], q_eo[:, 0:D], rz[:, 0:1])
            nc.vector.tensor_scalar_mul(
                qn[:, D : 2 * D], q_eo[:, D : 2 * D], rz[:, 1:2]
            )

            # out[tp,d] = sum_i E_even[i,tp]*qn_even[i,d] + E_odd[i,tp]*qn_odd[i,d]
            # NOTE: each c's (even, odd) accumulation pair must stay adjacent —
            # opening both column-halves' PSUM accumulation groups at once lets
            # the second start=True clear the shared bank row (silent corruption)
            out_ps = sc_pool.tile([128, TP], f32, name="out_ps", tag="sc")
            for c in range(2):
                nc.tensor.matmul(
                    out_ps[:, c * D : (c + 1) * D],
                    lhsT=e_sb[:, c * 128 : (c + 1) * 128],
                    rhs=qn[:, 0:D],
                    start=True,
                    stop=False,
                )
                nc.tensor.matmul(
                    out_ps[:, c * D : (c + 1) * D],
                    lhsT=e_sb[:, TP + c * 128 : TP + (c + 1) * 128],
                    rhs=qn[:, D : 2 * D],
                    start=False,
                    stop=True,
                )
            out_sb = consts.tile([128, TP], f32, name="out_sb", tag="out_sb")
            nc.vector.tensor_copy(out_sb, out_ps)
            nc.sync.dma_start(
                out_d.rearrange("(h t) d -> t h d", h=2),
                out_sb[:, :].rearrange("p (h d) -> p h d", h=2),
            )

        if repeat == 1:
            body()
        else:
            with tc.For_i(
                0,
                repeat,
                1,
                hint_engines=(
                    mybir.EngineType.PE,
                    mybir.EngineType.DVE,
                    mybir.EngineType.Activation,
                ),
            ):
                body()

    nc.compile()
    return nc


def get_nc(repeat=1, **kw):
    key = ("nc", repeat, tuple(sorted(kw.items())))
    if key not in _nc_cache:
        _nc_cache[key] = _build_nc(repeat, **kw)
    return _nc_cache[key]


last_results = None


def kernel(q, p, W, vd, _repeat=1, **_kw):
    global last_results
    from concourse.bass_utils import run_bass_kernel_spmd

    q = np.ascontiguousarray(np.asarray(q), dtype=np.float32)
    p = np.ascontiguousarray(np.asarray(p), dtype=np.float32)
    W = np.ascontiguousarray(np.asarray(W), dtype=np.float32)
    vd = np.ascontiguousarray(np.asarray(vd), dtype=np.float32)

    nc = get_nc(_repeat, **_kw)
    in_maps = [
        {"q": q[b], "p": p[b], "W": W, "vd": vd} for b in range(B)
    ]
    res = run_bass_kernel_spmd(nc, in_maps, core_ids=list(range(NCORES)))
    last_results = res
    return np.stack([r["out"] for r in res.results], axis=0)

